# revision 34
# baseline (speedup 1.0000x reference)
"""Trainium2 Bass kernel for batched tanh-query attention.

Per-batch computation (B=8, one batch per NeuronCore, pure data parallel):
    q = tanh(out_state)            [Q, H]    Q=K=2048, H=128
    S = q @ history.T              [Q, K]
    P = softmax(S, axis=K)
    attn = P @ history             [Q, H]

Flash-style, no HBM intermediates, computed in the transposed orientation
S_T[k, q] so the second matmul needs no transpose of P. Queries are
processed in 4 quarters of 512 columns; each quarter runs two software
phases that overlap across quarters:
  A(q): per kb-pair  S_T = ht[kb].T @ qT  (PE) -> exp FD=1024 (ACT, bf16)
        + two levels of bf16 pair-adds on DVE for the softmax denominator
  B(q): 16 accumulating  attn_T += hn[kb].T @ expS  matmuls + 4 ones-matmuls
        for d, emitted in two dense batches inside A(q+1) so PE runs them
        back-to-back while ACT keeps computing exps.
Epilogue per quarter (PE-transpose attn_T / d back to q-major, 1/d scale,
DMA out) is drained into later quarters' A phases.
"""

import sys

for _p in ("/opt/trn_rl_repo", "/opt/trn_rl_repo/concourse"):
    if _p not in sys.path:
        sys.path.insert(0, _p)

import numpy as np

N_CORES = 8
SEQ = 2048
H = 128
P = 128
T = SEQ // P          # 16 seq tiles
NQ = 4                # query quarters
QW = SEQ // NQ        # 512
QTPQ = QW // P        # 4 q-tiles per quarter
NPAIR = T // 2        # 8 kb-pairs per quarter

_CACHE = {}


def _build():
    from concourse import bacc, bass, masks, mybir, tile

    f32 = mybir.dt.float32
    bf16 = mybir.dt.bfloat16
    AF = mybir.ActivationFunctionType

    nc = bacc.Bacc("TRN2", target_bir_lowering=False, debug=False,
                   num_devices=N_CORES)
    os_d = nc.dram_tensor("out_state", (SEQ, H), f32, kind="ExternalInput")
    h_d = nc.dram_tensor("history", (SEQ, H), f32, kind="ExternalInput")
    a_d = nc.dram_tensor("attn", (SEQ, H), f32, kind="ExternalOutput")

    with tile.TileContext(nc) as tc:
        with (
            tc.tile_pool(name="const", bufs=1) as constp,
            tc.tile_pool(name="big", bufs=1) as bigp,
            tc.tile_pool(name="stage", bufs=2) as stagep,
            tc.tile_pool(name="work", bufs=4) as workp,
            tc.tile_pool(name="expool", bufs=11) as expool,
            tc.tile_pool(name="dtree", bufs=6) as dtreep,
            tc.tile_pool(name="ps", bufs=2, space=bass.MemorySpace.PSUM) as psp,
            tc.tile_pool(name="psacc", bufs=2, space=bass.MemorySpace.PSUM) as pacc,
            tc.tile_pool(name="psd", bufs=2, space=bass.MemorySpace.PSUM) as psd,
        ):
            id_f32 = constp.tile([P, P], f32, tag="idf")
            masks.make_identity(nc, id_f32[:])
            id_bf = constp.tile([P, P], bf16, tag="idb")
            masks.make_identity(nc, id_bf[:])
            ones_bf = constp.tile([P, P], bf16, tag="ones")
            nc.vector.memset(ones_bf[:], 1.0)

            # persistent bf16 operands
            hn = bigp.tile([P, T, P], bf16, tag="hn")    # [k_in, t, h] natural
            ht = bigp.tile([P, T, P], bf16, tag="ht")    # [h, t, k_in]
            qT = bigp.tile([P, T, P], bf16, tag="qT")    # [h, t, q_in]

            # ---- load + preprocess (chunked so compute starts early) ----
            os_f = stagep.tile([P, T, H], f32, tag="ldin")
            hn_f = stagep.tile([P, T, H], f32, tag="ldin")
            os_v = os_d[:].rearrange("(t p) h -> p t h", p=P)
            hn_v = h_d[:].rearrange("(t p) h -> p t h", p=P)
            # first chunks are small so the tanh/transpose chain starts early
            for lo, hi in ((0, 2), (2, 4), (4, 8), (8, 12), (12, 16)):
                sl = slice(lo, hi)
                nc.sync.dma_start(os_f[:, sl, :], os_v[:, sl, :])
                nc.sync.dma_start(hn_f[:, sl, :], hn_v[:, sl, :])

            q_nat = stagep.tile([P, T, H], bf16, tag="qnat")
            for lo, hi in ((0, 2), (2, 4), (4, 8)):
                sl = slice(lo, hi)
                nc.scalar.activation(q_nat[:, sl, :], os_f[:, sl, :], AF.Tanh)
                nc.vector.tensor_copy(hn[:, sl, :], hn_f[:, sl, :])

            def late_prep():
                for j in range(2, 4):
                    sl = slice(4 * j, 4 * (j + 1))
                    nc.scalar.activation(q_nat[:, sl, :], os_f[:, sl, :],
                                         AF.Tanh)
                    nc.vector.tensor_copy(hn[:, sl, :], hn_f[:, sl, :])

            # PE-transpose one [128,128] bf16 tile into a transposed layout
            def ptranspose(dst, src):
                tp = psd.tile([P, P], bf16, tag="dbc", name="tp")
                nc.tensor.transpose(tp[:], src, id_bf[:])
                nc.vector.tensor_copy(dst, tp[:])

            # aux work queue: input transposes now, epilogue tiles later
            aux = []

            def drain_aux(n):
                for _ in range(n):
                    if aux:
                        aux.pop(0)()

            def tp_job(kind, t):
                def job():
                    src = hn if kind == "h" else q_nat
                    dst = ht if kind == "h" else qT
                    ptranspose(dst[:, t, :], src[:, t, :])
                return job

            # upfront: tiles the first A-phase pairs need
            for t in range(QTPQ):
                ptranspose(qT[:, t, :], q_nat[:, t, :])
            for t in range(2):
                ptranspose(ht[:, t, :], hn[:, t, :])
            aux.extend(tp_job("h", t) for t in range(2, T))
            aux.extend(tp_job("q", t) for t in range(QTPQ, T))

            # ---- epilogue helper: one output q-tile of 128 rows ----
            def emit_epi(q, t, aT_sb, d_sb):
                dps = pacc.tile([P, 1], f32, tag="acc", name="dps")
                nc.tensor.transpose(dps[:], d_sb[0:1, P * t: P * (t + 1)],
                                    id_f32[0:1, 0:1])
                rc = workp.tile([P, 1], f32, tag="rc", name="rc")
                nc.vector.reciprocal(rc[:], dps[:])
                aps = pacc.tile([P, P], f32, tag="acc", name="aps")
                nc.tensor.transpose(aps[:], aT_sb[:, P * t: P * (t + 1)],
                                    id_f32[:])
                ot = workp.tile([P, P], f32, tag="ot", name="ot")
                nc.vector.tensor_scalar_mul(ot[:], aps[:], rc[:])
                row0 = q * QW + P * t
                nc.sync.dma_start(a_d[row0: row0 + P, :], ot[:])

            # ---- build per-quarter phase closures ----
            ex_tiles = [[] for _ in range(NQ)]
            lvl2s = [[] for _ in range(NQ)]
            accs = [None] * NQ
            dqs = [None] * NQ
            l1prev = [None] * NQ

            def emit_pair(q, p):
                if accs[q] is None:
                    accs[q] = pacc.tile([P, QW], f32, tag="acc",
                                        name=f"acc{q}")
                    dqs[q] = psd.tile([P, QW], f32, tag="dbc", name=f"dq{q}")
                kb0 = 2 * p
                st = psp.tile([P, 2 * QW], f32, tag="st", name="st")
                rhs = qT[:, QTPQ * q: QTPQ * (q + 1), :]
                nc.tensor.matmul(st[:, 0:QW], ht[:, kb0, :], rhs,
                                 start=True, stop=True)
                nc.tensor.matmul(st[:, QW:], ht[:, kb0 + 1, :], rhs,
                                 start=True, stop=True)
                ex = expool.tile([P, 2 * QW], bf16, tag="ex", name="ex")
                nc.scalar.activation(ex[:], st[:], AF.Exp)
                ex_tiles[q].append(ex)
                # d: in-tile pair add, then quad add (DVE, bf16)
                t1 = dtreep.tile([P, QW], bf16, tag="l1", name="t1")
                nc.vector.tensor_add(t1[:], ex[:, 0:QW], ex[:, QW:])
                if l1prev[q] is None:
                    l1prev[q] = t1
                else:
                    t2 = dtreep.tile([P, QW], bf16, tag="l2", name="t2")
                    nc.vector.tensor_add(t2[:], l1prev[q][:], t1[:])
                    l1prev[q] = None
                    lvl2s[q].append(t2)

            def emit_B(q, i):
                # batch i: 8 accumulating MM2s + two d quad matmuls
                for kb in range(8 * i, 8 * (i + 1)):
                    nc.tensor.matmul(
                        accs[q][:], hn[:, kb, :],
                        ex_tiles[q][kb // 2][:, QW * (kb % 2): QW * (kb % 2 + 1)],
                        start=(kb == 0), stop=(kb == T - 1))
                for j in (2 * i, 2 * i + 1):
                    nc.tensor.matmul(dqs[q][:], ones_bf[:], lvl2s[q][j][:],
                                     start=(j == 0), stop=(j == 3))
                if i == 1:
                    # move accumulators to SBUF, queue epilogue tiles
                    aT_sb = workp.tile([P, QW], f32, tag="atsb",
                                       name=f"aT{q}")
                    nc.vector.tensor_copy(aT_sb[:], accs[q][:])
                    d_sb = workp.tile([P, QW], f32, tag="dsb", name=f"d{q}")
                    nc.vector.tensor_copy(d_sb[:], dqs[q][:])
                    aux.extend(
                        (lambda t=t, a=aT_sb, d=d_sb, q=q: emit_epi(q, t, a, d))
                        for t in range(QTPQ))

            # ---- emission schedule ----
            for q in range(NQ):
                for p in range(NPAIR):
                    if q == 0 and p >= 1:
                        # ht transposes, one pair ahead of their consumer
                        drain_aux(2)
                    emit_pair(q, p)
                    if q == 0 and p == 2:
                        late_prep()
                    if q > 0 and p in (1, 3):
                        emit_B(q - 1, p // 2)
                    if q == NQ - 1 and p == 5:
                        emit_B(q, 0)
                    if p >= 4:
                        drain_aux(2)
            emit_B(NQ - 1, 1)
            while aux:
                aux.pop(0)()

    nc.compile()
    return nc


def _get_nc():
    if "nc" not in _CACHE:
        _CACHE["nc"] = _build()
    return _CACHE["nc"]


def _run(out_state, history, trace=False):
    from concourse.bass_utils import run_bass_kernel_spmd

    nc = _get_nc()
    out_state = np.ascontiguousarray(out_state, dtype=np.float32)
    history = np.ascontiguousarray(history, dtype=np.float32)
    in_maps = [
        {"out_state": out_state[b], "history": history[b]}
        for b in range(N_CORES)
    ]
    res = run_bass_kernel_spmd(nc, in_maps, core_ids=list(range(N_CORES)),
                               trace=trace)
    attn = np.stack([res.results[b]["attn"] for b in range(N_CORES)], axis=0)
    return attn.astype(np.float32), res


def kernel(out_state, history):
    attn, _ = _run(out_state, history)
    return attn


# revision 36
# speedup vs baseline: 1.0088x; 1.0088x over previous
"""Trainium2 Bass kernel for batched tanh-query attention.

Per-batch computation (B=8, one batch per NeuronCore, pure data parallel):
    q = tanh(out_state)            [Q, H]    Q=K=2048, H=128
    S = q @ history.T              [Q, K]
    P = softmax(S, axis=K)
    attn = P @ history             [Q, H]

Flash-style, no HBM intermediates, computed in the transposed orientation
S_T[k, q] so the second matmul needs no transpose of P. Queries are
processed in 4 quarters of 512 columns; each quarter runs two software
phases that overlap across quarters:
  A(q): per kb-pair  S_T = ht[kb].T @ qT  (PE) -> exp FD=1024 (ACT, bf16)
        + two levels of bf16 pair-adds on DVE for the softmax denominator
  B(q): 16 accumulating  attn_T += hn[kb].T @ expS  matmuls + 4 ones-matmuls
        for d, emitted in two dense batches inside A(q+1) so PE runs them
        back-to-back while ACT keeps computing exps.
Epilogue per quarter (PE-transpose attn_T / d back to q-major, 1/d scale,
DMA out) is drained into later quarters' A phases.
"""

import sys

for _p in ("/opt/trn_rl_repo", "/opt/trn_rl_repo/concourse"):
    if _p not in sys.path:
        sys.path.insert(0, _p)

import numpy as np

N_CORES = 8
SEQ = 2048
H = 128
P = 128
T = SEQ // P          # 16 seq tiles
NQ = 4                # query quarters
QW = SEQ // NQ        # 512
QTPQ = QW // P        # 4 q-tiles per quarter
NPAIR = T // 2        # 8 kb-pairs per quarter

_CACHE = {}


def _build():
    from concourse import bacc, bass, masks, mybir, tile

    f32 = mybir.dt.float32
    bf16 = mybir.dt.bfloat16
    AF = mybir.ActivationFunctionType

    nc = bacc.Bacc("TRN2", target_bir_lowering=False, debug=False,
                   num_devices=N_CORES)
    os_d = nc.dram_tensor("out_state", (SEQ, H), f32, kind="ExternalInput")
    h_d = nc.dram_tensor("history", (SEQ, H), f32, kind="ExternalInput")
    a_d = nc.dram_tensor("attn", (SEQ, H), f32, kind="ExternalOutput")

    with tile.TileContext(nc) as tc:
        with (
            tc.tile_pool(name="const", bufs=1) as constp,
            tc.tile_pool(name="big", bufs=1) as bigp,
            tc.tile_pool(name="stage", bufs=2) as stagep,
            tc.tile_pool(name="work", bufs=4) as workp,
            tc.tile_pool(name="expool", bufs=11) as expool,
            tc.tile_pool(name="dtree", bufs=6) as dtreep,
            tc.tile_pool(name="ps", bufs=2, space=bass.MemorySpace.PSUM) as psp,
            tc.tile_pool(name="psacc", bufs=2, space=bass.MemorySpace.PSUM) as pacc,
            tc.tile_pool(name="psd", bufs=2, space=bass.MemorySpace.PSUM) as psd,
        ):
            id_f32 = constp.tile([P, P], f32, tag="idf")
            masks.make_identity(nc, id_f32[:])
            id_bf = constp.tile([P, P], bf16, tag="idb")
            masks.make_identity(nc, id_bf[:])
            ones_bf = constp.tile([P, P], bf16, tag="ones")
            nc.vector.memset(ones_bf[:], 1.0)

            # persistent bf16 operands
            hn = bigp.tile([P, T, P], bf16, tag="hn")    # [k_in, t, h] natural
            ht = bigp.tile([P, T, P], bf16, tag="ht")    # [h, t, k_in]
            qT = bigp.tile([P, T, P], bf16, tag="qT")    # [h, t, q_in]

            # ---- load + preprocess (chunked so compute starts early) ----
            os_f = stagep.tile([P, T, H], f32, tag="ldin")
            hn_f = stagep.tile([P, T, H], f32, tag="ldin")
            os_v = os_d[:].rearrange("(t p) h -> p t h", p=P)
            hn_v = h_d[:].rearrange("(t p) h -> p t h", p=P)
            for j in range(4):
                sl = slice(4 * j, 4 * (j + 1))
                nc.sync.dma_start(os_f[:, sl, :], os_v[:, sl, :])
                nc.sync.dma_start(hn_f[:, sl, :], hn_v[:, sl, :])

            q_nat = stagep.tile([P, T, H], bf16, tag="qnat")
            for j in range(2):
                sl = slice(4 * j, 4 * (j + 1))
                nc.scalar.activation(q_nat[:, sl, :], os_f[:, sl, :], AF.Tanh)
                nc.vector.tensor_copy(hn[:, sl, :], hn_f[:, sl, :])

            def late_prep():
                for j in range(2, 4):
                    sl = slice(4 * j, 4 * (j + 1))
                    nc.scalar.activation(q_nat[:, sl, :], os_f[:, sl, :],
                                         AF.Tanh)
                    nc.vector.tensor_copy(hn[:, sl, :], hn_f[:, sl, :])

            # PE-transpose one [128,128] bf16 tile into a transposed layout
            def ptranspose(dst, src):
                tp = psd.tile([P, P], bf16, tag="dbc", name="tp")
                nc.tensor.transpose(tp[:], src, id_bf[:])
                nc.vector.tensor_copy(dst, tp[:])

            # aux work queue: input transposes now, epilogue tiles later
            aux = []

            def drain_aux(n):
                for _ in range(n):
                    if aux:
                        aux.pop(0)()

            def tp_job(kind, t):
                def job():
                    src = hn if kind == "h" else q_nat
                    dst = ht if kind == "h" else qT
                    ptranspose(dst[:, t, :], src[:, t, :])
                return job

            # upfront: tiles the first A-phase pairs need
            for t in range(QTPQ):
                ptranspose(qT[:, t, :], q_nat[:, t, :])
            for t in range(2):
                ptranspose(ht[:, t, :], hn[:, t, :])
            aux.extend(tp_job("h", t) for t in range(2, T))
            aux.extend(tp_job("q", t) for t in range(QTPQ, T))

            # ---- epilogue helper: one output q-tile of 128 rows ----
            def emit_epi(q, t, aT_sb, d_sb):
                dps = pacc.tile([P, 1], f32, tag="acc", name="dps")
                nc.tensor.transpose(dps[:], d_sb[0:1, P * t: P * (t + 1)],
                                    id_f32[0:1, 0:1])
                rc = workp.tile([P, 1], f32, tag="rc", name="rc")
                nc.vector.reciprocal(rc[:], dps[:])
                aps = pacc.tile([P, P], f32, tag="acc", name="aps")
                nc.tensor.transpose(aps[:], aT_sb[:, P * t: P * (t + 1)],
                                    id_f32[:])
                ot = workp.tile([P, P], f32, tag="ot", name="ot")
                nc.vector.tensor_scalar_mul(ot[:], aps[:], rc[:])
                row0 = q * QW + P * t
                nc.sync.dma_start(a_d[row0: row0 + P, :], ot[:])

            # ---- build per-quarter phase closures ----
            ex_tiles = [[] for _ in range(NQ)]
            lvl2s = [[] for _ in range(NQ)]
            accs = [None] * NQ
            dqs = [None] * NQ
            l1prev = [None] * NQ

            def emit_pair(q, p):
                if accs[q] is None:
                    accs[q] = pacc.tile([P, QW], f32, tag="acc",
                                        name=f"acc{q}")
                    dqs[q] = psd.tile([P, QW], f32, tag="dbc", name=f"dq{q}")
                kb0 = 2 * p
                st = psp.tile([P, 2 * QW], f32, tag="st", name="st")
                rhs = qT[:, QTPQ * q: QTPQ * (q + 1), :]
                nc.tensor.matmul(st[:, 0:QW], ht[:, kb0, :], rhs,
                                 start=True, stop=True)
                nc.tensor.matmul(st[:, QW:], ht[:, kb0 + 1, :], rhs,
                                 start=True, stop=True)
                ex = expool.tile([P, 2 * QW], bf16, tag="ex", name="ex")
                nc.scalar.activation(ex[:], st[:], AF.Exp)
                ex_tiles[q].append(ex)
                # d: in-tile pair add, then quad add (DVE, bf16)
                t1 = dtreep.tile([P, QW], bf16, tag="l1", name="t1")
                nc.vector.tensor_add(t1[:], ex[:, 0:QW], ex[:, QW:])
                if l1prev[q] is None:
                    l1prev[q] = t1
                else:
                    t2 = dtreep.tile([P, QW], bf16, tag="l2", name="t2")
                    nc.vector.tensor_add(t2[:], l1prev[q][:], t1[:])
                    l1prev[q] = None
                    lvl2s[q].append(t2)

            def emit_B(q, i):
                # batch i: 8 accumulating MM2s + two d quad matmuls
                for kb in range(8 * i, 8 * (i + 1)):
                    nc.tensor.matmul(
                        accs[q][:], hn[:, kb, :],
                        ex_tiles[q][kb // 2][:, QW * (kb % 2): QW * (kb % 2 + 1)],
                        start=(kb == 0), stop=(kb == T - 1))
                for j in (2 * i, 2 * i + 1):
                    nc.tensor.matmul(dqs[q][:], ones_bf[:], lvl2s[q][j][:],
                                     start=(j == 0), stop=(j == 3))
                if i == 1:
                    # move accumulators to SBUF, queue epilogue tiles
                    aT_sb = workp.tile([P, QW], f32, tag="atsb",
                                       name=f"aT{q}")
                    nc.vector.tensor_copy(aT_sb[:], accs[q][:])
                    d_sb = workp.tile([P, QW], f32, tag="dsb", name=f"d{q}")
                    nc.vector.tensor_copy(d_sb[:], dqs[q][:])
                    aux.extend(
                        (lambda t=t, a=aT_sb, d=d_sb, q=q: emit_epi(q, t, a, d))
                        for t in range(QTPQ))

            # ---- emission schedule ----
            for q in range(NQ):
                for p in range(NPAIR):
                    if q == 0 and p >= 1:
                        # ht transposes, one pair ahead of their consumer
                        drain_aux(2)
                    emit_pair(q, p)
                    if q == 0 and p == 2:
                        late_prep()
                    if q > 0 and p in (1, 3):
                        emit_B(q - 1, p // 2)
                    if q == NQ - 1 and p == 5:
                        emit_B(q, 0)
                    if p >= 4:
                        drain_aux(2)
            emit_B(NQ - 1, 1)
            while aux:
                aux.pop(0)()

    nc.compile()
    return nc


def _get_nc():
    if "nc" not in _CACHE:
        _CACHE["nc"] = _build()
    return _CACHE["nc"]


def _run(out_state, history, trace=False):
    from concourse.bass_utils import run_bass_kernel_spmd

    nc = _get_nc()
    out_state = np.ascontiguousarray(out_state, dtype=np.float32)
    history = np.ascontiguousarray(history, dtype=np.float32)
    in_maps = [
        {"out_state": out_state[b], "history": history[b]}
        for b in range(N_CORES)
    ]
    res = run_bass_kernel_spmd(nc, in_maps, core_ids=list(range(N_CORES)),
                               trace=trace)
    attn = np.stack([res.results[b]["attn"] for b in range(N_CORES)], axis=0)
    return attn.astype(np.float32), res


def kernel(out_state, history):
    attn, _ = _run(out_state, history)
    return attn
